# revision 1
# baseline (speedup 1.0000x reference)
"""Trainium2 Bass kernel for nn_Castro2025Model.

Contract: kernel(**inputs) takes FULL inputs {inputs:[8192,512,8] f32,
params_raw:[13] f32} and returns the FULL output [8192,512,4] f32.
Internally: data-parallel shard over the session axis across 8 NeuronCores.

Algorithm notes (validated vs the jax reference to ~3e-7 rel):
  The per-trial Q update with full error overwrite is exactly
      q[cc] <- c_t,   c_t = rv*(1+gamma) - gamma   (exact: rewards binary)
  followed by mean-mixing  q <- alpha_t*q' + beta_t*sum(q')*1.
  With per-chunk descaling  q~ = q / prod(alpha)  the recurrence becomes
      v_t   = overwrite(q~_{t-1}, cc_t, c~_t)          [copy_predicated]
      sig_t = sum_A(v_t)                               [tensor_reduce]
      q~_t  = v_t + rho_t * sig_t                      [scalar_tensor_tensor]
  which is 3-4 DVE ops per sequential step; everything else (c~, cum, tsls,
  softmax+lapse+log, bonus) is parallel over T in big tiles on ACT/GPSIMD/DVE,
  with lapse folded into ACT immediates:
      P = exp(z + ln(1-lapse));  S = sum_A P
      logits = ln(P + lapse/(4(1-lapse))*S) - ln(S/(1-lapse))
"""

import math
import numpy as np

A = 4
NCORES = 8
PART = 128


# ---------------------------------------------------------------- host math
def _host_params(params_raw: np.ndarray) -> dict:
    p = params_raw.astype(np.float64)

    def sp(x):
        return np.log1p(np.exp(-abs(x))) + max(x, 0.0)

    def sg(x):
        return 1.0 / (1.0 + np.exp(-x))

    return dict(
        beta_r=float(np.clip(sp(p[0]), 0.01, 20.0)),
        lapse=float(np.clip(sg(p[1]), 0.01, 0.99)),
        prior=float(np.clip(sp(p[2]), 0.01, 0.99)),
        alpha=float(np.clip(sg(p[3]), 0.01, 0.99)),
        decay=float(np.clip(sg(p[4]), 0.01, 0.99)),
        ab1=float(p[5]),
        ab2=float(p[6]),
        pers=float(sp(p[7])),
        sw=float(p[8]),
        gamma=float(sp(p[10])),
        temp=float(np.clip(sp(p[11]) + 1e-6, 1e-6, 100.0)),
        beta_p=float(sp(p[12])),
    )


def _host_schedule(pr: dict, T: int) -> dict:
    """Per-step constants: exploration decay chain (fp32-faithful), alpha/rho.

    The kernel stores q^ = k*q (k = beta_r/temp) directly: the per-step
    carry-over multiplies by alpha_t, and the overwrite value is
    c^ = k*alpha_t*c, so no running Phi product (and no descaling) exists.
    """
    e = np.empty(T, np.float64)
    x = np.float32(pr["alpha"])
    for t in range(T):
        x = np.float32(x * np.float32(1.0 - 1e-3))
        e[t] = float(x)
    alph = pr["decay"] * (1.0 - e)          # alpha_t
    rho = e / (4.0 * (1.0 - e))             # rho_t
    k = pr["beta_r"] / pr["temp"]
    # constant z-shift keeping exp() in range (cancels exactly in logits)
    zbound = k * max(1.0, pr["gamma"], pr["prior"]) \
        + pr["beta_p"] * math.log(513.0)
    zshift = max(0.0, zbound - 60.0)
    return dict(e=e, alph=alph, rho=rho, k=k, zshift=zshift)


# ---------------------------------------------------------------- program
def build_program(pr: dict, B_core: int, T: int, Tc: int,
                  copy_eng: str = 'vector', split_big: bool = False):
    """Build the per-core Bass program (SPMD across cores)."""
    import concourse.bass as bass
    import concourse.bacc as bacc
    import concourse.mybir as mybir
    import concourse.tile as tile

    f32 = mybir.dt.float32
    AL = mybir.AluOpType
    AF = mybir.ActivationFunctionType
    AX = mybir.AxisListType

    S = B_core // PART                       # sessions per partition
    NCH = T // Tc
    sch = _host_schedule(pr, T)
    rho = sch["rho"]
    alph = sch["alph"]
    k = sch["k"]

    lapse = pr["lapse"]
    ln1ml = math.log(1.0 - lapse) - sch["zshift"]
    lam2 = lapse / (4.0 * (1.0 - lapse))
    inv1ml = 1.0 / (1.0 - lapse)
    pers, sw, ab1, ab2 = pr["pers"], pr["sw"], pr["ab1"], pr["ab2"]
    prior = pr["prior"]
    beta_p = pr["beta_p"]

    nc = bacc.Bacc()
    x = nc.dram_tensor("x", [B_core, T, 2 * A], f32, kind="ExternalInput")
    # host vectors replicated across partitions: [w1, w2] (c^ coefficients)
    hv = nc.dram_tensor("hv", [PART, 2 * T], f32, kind="ExternalInput")
    # scaled identity matrices for PE accumulation: [I, beta_p*I, ab1*I, ab2*I]
    hm = nc.dram_tensor("hm", [PART, 4 * PART], f32, kind="ExternalInput")
    y = nc.dram_tensor("y", [B_core, T, A], f32, kind="ExternalOutput")

    xv = x.rearrange("(p s) t c -> p s t c", p=PART)      # [128,S,T,8]
    yv = y.rearrange("(p s) t j -> p s t j", p=PART)      # [128,S,T,4]

    def regconst(v):
        v = float(v)
        if (f32, v) not in nc.const_aps.aps:
            th = nc.alloc_sbuf_tensor(
                f"uconst_{len(nc.const_aps.aps)}", [PART, 1], f32)
            nc.gpsimd.memset(th.ap(), v)
            nc.const_aps.aps[(f32, v)] = th.ap()

    with tile.TileContext(nc) as tc:
        regconst(ln1ml)
        with (
            tc.tile_pool(name="const", bufs=1) as cstp,
            tc.tile_pool(name="ri", bufs=4) as rip,
            tc.tile_pool(name="qh", bufs=4) as qhp,
            tc.tile_pool(name="sig", bufs=2) as sigp,
            tc.tile_pool(name="ctl", bufs=2) as ctlp,
            tc.tile_pool(name="cum", bufs=2) as cump,
            tc.tile_pool(name="sm", bufs=2) as smp,
            tc.tile_pool(name="big", bufs=1) as bigp,
            tc.tile_pool(name="big2", bufs=(2 if split_big else 1)) as bigp2,
            tc.tile_pool(name="out", bufs=1) as outp,
            tc.tile_pool(name="ps", bufs=1, space="PSUM") as psp,
        ):
            # constants
            hvt = cstp.tile([PART, 2 * T], f32, tag="hv")
            nc.sync.dma_start(hvt.rearrange("p (r t) -> p r t", r=2),
                              hv.rearrange("p (r t) -> p r t", r=2))
            w1 = hvt[:, 0:T]
            w2 = hvt[:, T:2 * T]
            neg1 = cstp.tile([PART, S], f32, tag="neg1")
            nc.vector.memset(neg1[:, :], -1.0)
            zer = cstp.tile([PART, Tc], f32, tag="zer")
            nc.vector.memset(zer[:, :], 0.0)
            ccar = cstp.tile([PART, 2 * S * A], f32, tag="ccar")

            prev = dict(ri=None, qh=None, code=None, tsls=None, cum=None)

            for ck in range(NCH):
                t0 = ck * Tc
                # ---------- load input chunk: layout (s, t, ch) ----------
                ri = rip.tile([PART, S * Tc * 8], f32, tag="ri")
                riv = ri.rearrange("p (s t c) -> p s t c", s=S, t=Tc, c=8)
                nc.sync.dma_start(riv[:, :, :, :], xv[:, :, t0:t0 + Tc, :])

                def a_sj(trel):                       # [128,S,A] mask at t
                    return riv[:, :, trel, 0:A]

                # ---------- c~ chunk: layout (s, t) ----------
                ctl = ctlp.tile([PART, S * Tc], f32, tag="ctl")
                ctlv = ctl.rearrange("p (s t) -> p s t", s=S)
                rvv = riv[:, :, :, A]                 # [128,S,Tc] rewards
                w1b = w1[:, t0:t0 + Tc].unsqueeze(1).broadcast_to([PART, S, Tc])
                w2b = w2[:, t0:t0 + Tc].unsqueeze(1).broadcast_to([PART, S, Tc])
                nc.vector.tensor_tensor(out=ctlv, in0=rvv, in1=w1b, op=AL.mult)
                nc.vector.tensor_tensor(out=ctlv, in0=ctlv, in1=w2b, op=AL.subtract)

                # ---------- phase 1: sequential steps ----------
                qh = qhp.tile([PART, S * Tc * A], f32, tag="qh")
                qhv = qh.rearrange("p (s t j) -> p s t j", s=S, t=Tc)
                sig = sigp.tile([PART, S * Tc], f32, tag="sig")
                sigv = sig.rearrange("p (s t) -> p s t", s=S)
                for trel in range(Tc):
                    t = t0 + trel
                    dst = qhv[:, :, trel, :]          # [128,S,A]
                    if t == 0:
                        nc.vector.memset(dst, float(k * alph[0] * prior))
                    else:
                        src = (prev["qh"][:, :, Tc - 1, :] if trel == 0
                               else qhv[:, :, trel - 1, :])
                        if copy_eng == 'scalar':
                            nc.scalar.mul(dst, src, float(alph[t]))
                        else:
                            nc.vector.tensor_scalar_mul(
                                dst, src, float(alph[t]))
                    cbc = ctlv[:, :, trel].unsqueeze(2).broadcast_to([PART, S, A])
                    nc.vector.copy_predicated(
                        out=dst, mask=a_sj(trel).bitcast(mybir.dt.int32),
                        data=cbc)
                    nc.vector.tensor_reduce(
                        out=sigv[:, :, trel], in_=dst, axis=AX.X, op=AL.add)
                    sbc = sigv[:, :, trel].unsqueeze(2).broadcast_to([PART, S, A])
                    nc.vector.scalar_tensor_tensor(
                        out=dst, in0=sbc, scalar=float(rho[t]), in1=dst,
                        op0=AL.mult, op1=AL.add)

                # ---------- cum scans (t-major views into cum tile) ----------
                cum = cump.tile([PART, S * Tc * A], f32, tag="cum")
                cumv = cum.rearrange("p (s t j) -> p s t j", s=S, t=Tc)
                for s in range(S):
                    for j in range(A):
                        d0 = riv[:, s, :, j]          # [128,Tc]
                        dstc = cumv[:, s, :, j]       # [128,Tc] strided
                        cc0 = ((ck + 1) % 2) * S * A
                        init = (0.0 if ck == 0 else
                                ccar[:, cc0 + s * A + j:cc0 + s * A + j + 1])
                        nc.vector.tensor_tensor_scan(
                            out=dstc, data0=d0, data1=zer[:, :], initial=init,
                            op0=AL.add, op1=AL.add)

                cc1 = (ck % 2) * S * A
                nc.vector.tensor_copy(
                    out=ccar[:, cc1:cc1 + S * A].rearrange(
                        "p (s j) -> p s j", s=S),
                    in_=cumv[:, :, Tc - 1, :])

                # ---------- code / same / tsls / G ----------
                code = smp.tile([PART, S * Tc], f32, tag="code")
                codev = code.rearrange("p (s t) -> p s t", s=S)
                tmp = smp.tile([PART, S * Tc], f32, tag="smtmp")
                tmpv = tmp.rearrange("p (s t) -> p s t", s=S)
                nc.vector.scalar_tensor_tensor(
                    out=tmpv, in0=riv[:, :, :, 1], scalar=2.0, in1=riv[:, :, :, 0],
                    op0=AL.mult, op1=AL.add)
                nc.vector.scalar_tensor_tensor(
                    out=codev, in0=riv[:, :, :, 3], scalar=3.0, in1=tmpv,
                    op0=AL.mult, op1=AL.add)
                same = smp.tile([PART, S * Tc], f32, tag="same")
                samev = same.rearrange("p (s t) -> p s t", s=S)
                nc.vector.tensor_tensor(
                    out=samev[:, :, 1:], in0=codev[:, :, 1:],
                    in1=codev[:, :, 0:Tc - 1], op=AL.is_equal)
                carry = (neg1[:, :].unsqueeze(2) if ck == 0
                         else prev["code"][:, :, Tc - 1].unsqueeze(2))
                nc.vector.tensor_tensor(
                    out=samev[:, :, 0:1], in0=codev[:, :, 0:1], in1=carry,
                    op=AL.is_equal)
                tsls = smp.tile([PART, S * Tc], f32, tag="tsls")
                tslsv = tsls.rearrange("p (s t) -> p s t", s=S)
                for s in range(S):
                    init = (0.0 if ck == 0
                            else prev["tsls"][:, s, Tc - 1].unsqueeze(1))
                    nc.vector.tensor_tensor_scan(
                        out=tslsv[:, s], data0=samev[:, s], data1=samev[:, s],
                        initial=init, op0=AL.mult, op1=AL.add)
                # G = sw + (pers-sw)*same + ln(1+tsls)
                gv = tmpv                              # reuse tmp as G
                nc.scalar.activation(out=gv, in_=tslsv, func=AF.Ln, bias=1.0)
                gate = smp.tile([PART, S * Tc], f32, tag="gate")
                gatev = gate.rearrange("p (s t) -> p s t", s=S)
                nc.vector.tensor_scalar(
                    out=gatev, in0=samev, scalar1=pers - sw, scalar2=sw,
                    op0=AL.mult, op1=AL.add)
                nc.vector.tensor_tensor(out=gv, in0=gv, in1=gatev, op=AL.add)

                # -------- phase 2 big passes: 3-dim APs [p, (s t), j] --------
                ST = S * Tc
                a43 = ri.rearrange("p (st c) -> p st c", c=8)[:, :, 0:A]
                z = bigp2.tile([PART, ST * A], f32, tag="z")
                z3 = z.rearrange("p (st j) -> p st j", j=A)
                # z = q^ + beta_p * ln(1+cum)   (q^ already carries k)
                nc.scalar.activation(out=z[:, :], in_=cum[:, :],
                                     func=AF.Ln, bias=1.0)
                nc.scalar.mul(z[:, :], z[:, :], beta_p)
                nc.gpsimd.tensor_tensor(
                    out=z[:, :], in0=z[:, :], in1=qh[:, :], op=AL.add)
                # P = exp(z + ln(1-lapse)); S = sum_A P
                P = bigp.tile([PART, ST * A], f32, tag="P")
                P3 = P.rearrange("p (st j) -> p st j", j=A)
                nc.scalar.activation(out=P[:, :], in_=z[:, :], func=AF.Exp,
                                     bias=ln1ml)
                Ssum = sigp.tile([PART, ST], f32, tag="Ssum")
                nc.vector.tensor_reduce(
                    out=Ssum[:, :], in_=P3, axis=AX.X, op=AL.add)
                # g = P + lam2*S ; lg = ln(g); ls2 = ln(S/(1-lapse))
                slam = sigp.tile([PART, ST], f32, tag="slam")
                nc.scalar.mul(slam[:, :], Ssum[:, :], lam2)
                sb = slam[:, :].unsqueeze(2).broadcast_to([PART, ST, A])
                g3 = P3                                 # g/lg in place of P
                nc.gpsimd.tensor_tensor(out=g3, in0=sb, in1=P3, op=AL.add)
                nc.scalar.activation(out=P[:, :], in_=P[:, :], func=AF.Ln)
                ls2 = sigp.tile([PART, ST], f32, tag="ls2")
                nc.scalar.activation(out=ls2[:, :], in_=Ssum[:, :], func=AF.Ln,
                                     scale=inv1ml)
                # f4 = lg - ls2
                lsb = ls2[:, :].unsqueeze(2).broadcast_to([PART, ST, A])
                f4_3 = z3                               # reuse z tile
                nc.gpsimd.tensor_tensor(out=f4_3, in0=g3, in1=lsb,
                                        op=AL.subtract)
                # f1 = G*a  (Pool); rest of the bonus sum on PE into PSUM:
                #   out = f4 + f1 + ab1*a_prev + ab2*rot2(a)
                Gb = tmp[:, :].unsqueeze(2).broadcast_to([PART, ST, A])
                f1f = cum
                f1_3 = cum.rearrange("p (st j) -> p st j", j=A)
                nc.gpsimd.tensor_tensor(out=f1_3, in0=Gb, in1=a43, op=AL.mult)
                f4f = z
                # rotated / t-shifted copies of a via SBUF->SBUF DMA
                ri3 = ri.rearrange("p (st c) -> p st c", c=8)
                arot = bigp.tile([PART, ST * A], f32, tag="arot")
                ar3 = arot.rearrange("p (st j) -> p st j", j=A)
                nc.scalar.mul(ar3[:, :, 2:4], ri3[:, :, 0:2], ab2)
                nc.scalar.mul(ar3[:, :, 0:2], ri3[:, :, 2:4], ab2)
                ashf = bigp.tile([PART, ST * A], f32, tag="ashf")
                as3 = ashf.rearrange("p (st j) -> p st j", j=A)
                as4 = ashf.rearrange("p (s t j) -> p s t j", s=S, t=Tc)
                # (s t)-merged shift by one trial; per-session first rows
                # bleed from the previous session and are then overwritten
                nc.scalar.mul(as3[:, 1:, :], ri3[:, 0:ST - 1, 0:A], ab1)
                if ck == 0:
                    nc.vector.memset(as4[:, :, 0, :], 0.0)
                else:
                    nc.vector.tensor_scalar_mul(
                        as4[:, :, 0, :],
                        prev["ri"].rearrange("p (s t c) -> p s t c",
                                             s=S, t=Tc, c=8)[
                            :, :, Tc - 1, 0:A], ab1)
                nc.gpsimd.tensor_tensor(out=f4f[:, :], in0=f4f[:, :],
                                        in1=f1f[:, :], op=AL.add)
                nc.gpsimd.tensor_tensor(out=f4f[:, :], in0=f4f[:, :],
                                        in1=ashf[:, :], op=AL.add)
                ot = outp.tile([PART, ST * A], f32, tag="ot")
                nc.gpsimd.tensor_tensor(out=ot[:, :], in0=f4f[:, :],
                                        in1=arot[:, :], op=AL.add)
                ost = ot.rearrange("p (s t j) -> p s t j", s=S, t=Tc)
                nc.sync.dma_start(yv[:, :, t0:t0 + Tc, :], ost)

                prev = dict(ri=ri, qh=qhv, code=codev, tsls=tslsv, cum=cumv)

    nc.compile()
    return nc


def make_hv(pr: dict, sch: dict, T: int) -> np.ndarray:
    ka = sch["k"] * sch["alph"]              # k * alpha_t
    hvrow = np.concatenate([
        ((1.0 + pr["gamma"]) * ka).astype(np.float32),
        (pr["gamma"] * ka).astype(np.float32),
    ])
    return np.broadcast_to(hvrow, (PART, 2 * T)).copy()


def make_hm(pr: dict) -> np.ndarray:
    eye = np.eye(PART, dtype=np.float32)
    return np.concatenate(
        [eye, pr["beta_p"] * eye, pr["ab1"] * eye, pr["ab2"] * eye],
        axis=1).copy()


# ---------------------------------------------------------------- entry
def kernel(inputs: np.ndarray, params_raw: np.ndarray) -> np.ndarray:
    from concourse import bass_utils

    B, T = inputs.shape[0], inputs.shape[1]
    B_core = B // NCORES
    Tc = 64 if T % 64 == 0 else T
    pr = _host_params(np.asarray(params_raw))
    sch = _host_schedule(pr, T)

    nc = build_program(pr, B_core, T, Tc, split_big=True)

    hv = make_hv(pr, sch, T)

    xs = np.ascontiguousarray(np.asarray(inputs, np.float32))
    hm = make_hm(pr)
    in_maps = [
        {"x": xs[c * B_core:(c + 1) * B_core], "hv": hv, "hm": hm}
        for c in range(NCORES)
    ]
    res = bass_utils.run_bass_kernel_spmd(
        nc, in_maps, core_ids=list(range(NCORES)))
    return np.concatenate([r["y"] for r in res.results], axis=0)

